# revision 14
# baseline (speedup 1.0000x reference)
"""Causal squeeze-excite 1d on 8 TRN2 NeuronCores.

Reference computation (per batch b):
    y = causal_ema(x)                      # y[t] = (1-a) y[t-1] + a x[t], y[0] = x[0]
    h = relu(w1 @ y[:, t] + b1)            # (32,)  per time step
    g = sigmoid(w2 @ h + b2)               # (512,) per time step
    out[:, t] = x[:, t] * g
Sharding: data-parallel over batch; core i gets x[2i:2i+2].

Structure (v2, fp16 IO):
  - x/out/weights travel as fp16: halves HBM traffic (the kernel is
    DMA-bound); fp16's 2^-11 relative rounding is far inside tolerance.
    Host converts + lays DRAM out as [128p, b, chunk, cb, t] so every
    load is 128 descriptors x 8 KB contiguous.
  - EMA commutes with the channel projection: w1 @ ema(x) == ema((a*w1) @ x),
    so the DVE scan runs on a 32-row projected sequence, not [512, T].
  - Both batches stack in PSUM partitions (b0 rows 0-31, b1 rows 32-63 via
    PE tile_position), so ONE scan / ONE relu instruction covers both
    batches -- the scan is sequential per column, rows ride free.
  - b1 rides the DVE relu (fused add+max tensor_scalar); b2 rides the
    sigmoid ACTIVATE's per-partition bias.  No ones-row needed.
  - The gate multiply writes a fresh fp16 tile (not in-place) aiming for
    the DVE 2x packed 16-bit path; stores stream per cb-pair so the tail
    store starts right after its sigmoid.
  - All x loads AND stores issue on the Sync ring (HWDGE); weights/consts
    load via the Scalar ring so the first x load starts at t=0.
"""

import numpy as np
from contextlib import ExitStack

import concourse.bass as bass
import concourse.bacc as bacc
import concourse.tile as tile
import concourse.mybir as mybir
from concourse.bass_utils import run_bass_kernel_spmd

F32 = mybir.dt.float32
F16 = mybir.dt.float16

N_CORES = 8
B, C, T = 16, 512, 4096
CSQ = 32          # squeeze dim
P = 128           # SBUF partitions
NCB = C // P      # channel blocks (4)
B_LOC = B // N_CORES          # batches per core (2)
M2 = B_LOC * CSQ  # stacked mm1 output rows (64)
Tc = 1024         # time chunk
NCI = T // Tc     # DRAM chunk blocks (4)
TS = 512          # matmul / scan sub-tile (one PSUM bank)
PREF = 2          # load prefetch distance, in chunks
# Time chunks: the final 1024 columns run as two 512-col chunks so the
# last chunk's serial mm1->scan->relu->mm2->sigmoid->gate chain is
# half as deep.
CHUNKS = [(0, 1024), (1024, 1024), (2048, 1024), (3072, 512), (3584, 512)]
NTH = len(CHUNKS)


def build_nc(B_loc, cw, C_=C, T_=T):
    assert B_loc == B_LOC
    d = 1.0 - 1.0 / cw
    assert sum(c[1] for c in CHUNKS) == T_ and all(
        c % TS == 0 for _, c in CHUNKS)

    nc = bacc.Bacc(trn_type="TRN2")
    # x/out DRAM layout: [p, b, ci, cb, t] (fp16).  One load per (b, ci)
    # is 128 x 8KB contiguous; one store per (b, ci, cb-pair) is 128 x 4KB.
    xin = nc.declare_dram_parameter("x", [P, B_loc * NCI * NCB * Tc], F16,
                                    isOutput=False)
    w1e = nc.declare_dram_parameter("w1e", [P, NCB * CSQ], F16, isOutput=False)
    w2d = nc.declare_dram_parameter("w2d", [M2, C_], F16, isOutput=False)
    b1d = nc.declare_dram_parameter("b1d", [M2, 1], F32, isOutput=False)
    b2e = nc.declare_dram_parameter("b2e", [P, NCB], F32, isOutput=False)
    out = nc.declare_dram_parameter("out", [P, B_loc * NCI * NCB * Tc], F16,
                                    isOutput=True)

    xv = xin.rearrange("p (b ci cb t) -> p b ci cb t", b=B_loc, ci=NCI, cb=NCB)
    ov = out.rearrange("p (b ci cb t) -> p b ci cb t", b=B_loc, ci=NCI, cb=NCB)

    with ExitStack() as ctx:
        tc = ctx.enter_context(tile.TileContext(nc))
        const = ctx.enter_context(tc.tile_pool(name="const", bufs=1))
        xpool = ctx.enter_context(
            tc.tile_pool(name="xp", bufs=2 * (PREF + 1) + 1))
        opool = ctx.enter_context(tc.tile_pool(name="op", bufs=4))
        gpool = ctx.enter_context(tc.tile_pool(name="gp", bufs=4))
        upool = ctx.enter_context(tc.tile_pool(name="up", bufs=3))
        hpool = ctx.enter_context(tc.tile_pool(name="hp", bufs=3))
        cpool = ctx.enter_context(tc.tile_pool(name="cp", bufs=2))
        php = ctx.enter_context(tc.tile_pool(name="php", bufs=4, space="PSUM"))
        pgp = ctx.enter_context(tc.tile_pool(name="pgp", bufs=2, space="PSUM"))

        # Consts ride the Scalar HWDGE ring so the Sync ring starts on x
        # immediately.
        w1_t = const.tile([P, NCB * CSQ], F16, tag="w1e")
        nc.scalar.dma_start(w1_t[:], w1e[:])
        w2_t = const.tile([M2, C_], F16, tag="w2d")
        nc.scalar.dma_start(w2_t[:], w2d[:])
        b1_t = const.tile([M2, 1], F32, tag="b1d")
        nc.scalar.dma_start(b1_t[:], b1d[:])
        b2_t = const.tile([P, NCB], F32, tag="b2e")
        nc.scalar.dma_start(b2_t[:], b2e[:])
        dconst = const.tile([M2, TS], F32, tag="dconst")
        nc.vector.memset(dconst[:], d)

        xts = {}

        def emit_loads(ci):
            t0, tcc = CHUNKS[ci]
            dci, dt0 = divmod(t0, Tc)
            tiles = []
            for b in range(B_loc):
                xt = xpool.tile([P, NCB * Tc], F16, tag="x", name=f"x{b}_{ci}")
                xw = xt[:].rearrange("p (cb t) -> p cb t", cb=NCB)
                nc.sync.dma_start(
                    xw[:, :, 0:tcc],
                    xv[:, b, dci, :, dt0:dt0 + tcc])
                tiles.append(xt)
            xts[ci] = tiles

        for ci in range(min(PREF, NTH)):
            emit_loads(ci)

        ph_pre = {}

        def phase1(ci):
            # mm1 for chunk ci, both batches stacked into one PSUM tile
            # (b0 -> rows 0-31, b1 -> rows 32-63 via PE tile placement).
            # Emitted one chunk ahead so the PE never sits behind a
            # relu-blocked mm2 while independent mm1 work exists.
            _, tcc = CHUNKS[ci]
            nts = tcc // TS
            xws_ = [xt[:].rearrange("p (cb t) -> p cb t", cb=NCB)
                    for xt in xts[ci]]
            phs = [None] * nts
            for ts in range(nts):
                ph = php.tile([M2, TS], F32, tag="ph")
                for b in range(B_loc):
                    dst = ph[b * CSQ:(b + 1) * CSQ, :]
                    for cb in range(NCB):
                        nc.tensor.matmul(
                            dst,
                            w1_t[:, cb * CSQ:(cb + 1) * CSQ],
                            xws_[b][:, cb, ts * TS:(ts + 1) * TS],
                            start=(cb == 0), stop=(cb == NCB - 1))
                phs[ts] = ph
            ph_pre[ci] = phs

        phase1(0)
        carry = None
        for th in range(NTH):
            if th + PREF < NTH:
                emit_loads(th + PREF)
            if th + 1 < NTH:
                phase1(th + 1)
            t0, tcc = CHUNKS[th]
            dci, dt0 = divmod(t0, Tc)
            nts = tcc // TS
            xb = xts.pop(th)
            phs = ph_pre.pop(th)
            # Phase 2: one scan per TS sub-tile + one fused relu per chunk,
            # covering BOTH batches (stacked rows).
            ut = upool.tile([M2, Tc], F32, tag="u")
            for ts in range(nts):
                if th == 0 and ts == 0:
                    # u_0 = cw * p_0 makes y[0] = x[0] exact.
                    init = cpool.tile([M2, 1], F32, tag="c")
                    nc.scalar.mul(init[:], phs[ts][:, 0:1], float(cw))
                    init_ap = init[:]
                else:
                    init_ap = carry
                nc.vector.tensor_tensor_scan(
                    ut[:, ts * TS:(ts + 1) * TS], dconst[:],
                    phs[ts][:], init_ap,
                    mybir.AluOpType.mult, mybir.AluOpType.add)
                carry = ut[:, (ts + 1) * TS - 1:(ts + 1) * TS]
            # Fused (u + b1) -> max(., 0) on the DVE keeps ACT free for
            # sigmoids (ACT is the busiest compute engine at fp16 IO).
            ht = hpool.tile([M2, Tc], F16, tag="h")
            nc.vector.tensor_scalar(
                ht[:, 0:tcc], ut[:, 0:tcc], b1_t[:], 0.0,
                mybir.AluOpType.add, mybir.AluOpType.max)
            # Phase 3: mm2 + sigmoid per (b, cb); all time sub-tiles of
            # the chunk land in one PSUM tile -> one sigmoid each, with
            # b2 riding the ACTIVATE's per-partition bias.
            gts = [gpool.tile([P, NCB * Tc], F16, tag="g", name=f"g{b}")
                   for b in range(B_loc)]
            gws = [g[:].rearrange("p (cb t) -> p cb t", cb=NCB) for g in gts]
            for b in range(B_loc):
                hsl = ht[b * CSQ:(b + 1) * CSQ, :]
                for cb in range(NCB):
                    pg = pgp.tile([P, Tc], F32, tag="pg")
                    wsl = w2_t[b * CSQ:(b + 1) * CSQ, cb * P:(cb + 1) * P]
                    for ts in range(nts):
                        nc.tensor.matmul(
                            pg[:, ts * TS:(ts + 1) * TS], wsl,
                            hsl[:, ts * TS:(ts + 1) * TS],
                            start=True, stop=True)
                    nc.scalar.activation(
                        gws[b][:, cb, 0:tcc], pg[:, 0:tcc],
                        mybir.ActivationFunctionType.Sigmoid,
                        bias=b2_t[:, cb:cb + 1])
            # Phase 4: gate multiply into a fresh fp16 tile (all-16-bit,
            # step-1, 4B-aligned -> DVE packed rate), one piece per
            # cb-pair so each store can stream as soon as its half is
            # gated.  Stores stay on the Sync ring with the loads.
            for b in range(B_loc):
                ot = opool.tile([P, NCB * Tc], F16, tag="o", name=f"o{b}")
                ow = ot[:].rearrange("p (cb t) -> p cb t", cb=NCB)
                xw = xb[b][:].rearrange("p (cb t) -> p cb t", cb=NCB)
                for cbp in range(0, NCB, 2):
                    nc.vector.tensor_mul(
                        ow[:, cbp:cbp + 2, 0:tcc],
                        xw[:, cbp:cbp + 2, 0:tcc],
                        gws[b][:, cbp:cbp + 2, 0:tcc])
                    nc.sync.dma_start(
                        ov[:, b, dci, cbp:cbp + 2, dt0:dt0 + tcc],
                        ow[:, cbp:cbp + 2, 0:tcc])
    nc.compile()
    return nc


def make_in_maps(x, w1, b1, w2, b2, cw, n_cores=N_CORES):
    """Host-side shard + weight prep. Returns per-core input maps."""
    a = 1.0 / cw
    C_ = w2.shape[0]
    b_loc = x.shape[0] // n_cores

    w1sT = (np.asarray(w1) * a).T.astype(np.float32)      # [C, CSQ]
    w1e = np.empty((P, NCB * CSQ), dtype=np.float16)
    for cb in range(NCB):
        w1e[:, cb * CSQ:(cb + 1) * CSQ] = w1sT[cb * P:(cb + 1) * P, :]

    w2d = np.empty((M2, C_), dtype=np.float16)
    for b in range(b_loc):
        w2d[b * CSQ:(b + 1) * CSQ, :] = np.asarray(w2).T

    b1d = np.empty((M2, 1), dtype=np.float32)
    for b in range(b_loc):
        b1d[b * CSQ:(b + 1) * CSQ, 0] = np.asarray(b1)

    b2e = np.asarray(b2).astype(np.float32).reshape(NCB, P).T.copy()

    # [B, C, T] -> per-core [P, b, ci, cb, t] fp16 (see build_nc).
    x16 = np.asarray(x).astype(np.float16)
    x16 = x16.reshape(n_cores, b_loc, NCB, P, NCI, Tc)
    x16 = np.ascontiguousarray(x16.transpose(0, 3, 1, 4, 2, 5))
    x16 = x16.reshape(n_cores, P, b_loc * NCI * NCB * Tc)

    return [
        {"x": x16[i], "w1e": w1e, "w2d": w2d, "b1d": b1d, "b2e": b2e}
        for i in range(n_cores)
    ]


def unshard_out(results, n_cores=N_CORES, b_loc=B_LOC):
    """Per-core [P, b*ci*cb*t] fp16 -> full [B, C, T] fp32."""
    o = np.stack([r["out"] for r in results], axis=0)
    o = o.reshape(n_cores, P, b_loc, NCI, NCB, Tc)
    o = o.transpose(0, 2, 4, 1, 3, 5)          # [core, b, cb, p, ci, t]
    return np.ascontiguousarray(o).reshape(B, C, T).astype(np.float32)


_NC_CACHE = {}


def kernel(x, w1, b1, w2, b2, context_window):
    cw = int(context_window)
    x = np.asarray(x)
    key = (cw, x.shape)
    if key not in _NC_CACHE:
        _NC_CACHE[key] = build_nc(x.shape[0] // N_CORES, cw)
    nc = _NC_CACHE[key]
    in_maps = make_in_maps(
        np.asarray(x), np.asarray(w1), np.asarray(b1),
        np.asarray(w2), np.asarray(b2), cw)
    res = run_bass_kernel_spmd(nc, in_maps, core_ids=list(range(N_CORES)))
    return unshard_out(res.results)


# revision 17
# speedup vs baseline: 1.1291x; 1.1291x over previous
"""Causal squeeze-excite 1d on 8 TRN2 NeuronCores.

Reference computation (per batch b):
    y = causal_ema(x)                      # y[t] = (1-a) y[t-1] + a x[t], y[0] = x[0]
    h = relu(w1 @ y[:, t] + b1)            # (32,)  per time step
    g = sigmoid(w2 @ h + b2)               # (512,) per time step
    out[:, t] = x[:, t] * g
Sharding: data-parallel over batch; core i gets x[2i:2i+2].

Structure (v2, fp16 IO):
  - x/out/weights travel as fp16: halves HBM traffic (the kernel is
    DMA-bound); fp16's 2^-11 relative rounding is far inside tolerance.
    Host converts + lays DRAM out as [128p, b, chunk, cb, t] so every
    load is 128 descriptors x 8 KB contiguous.
  - EMA commutes with the channel projection: w1 @ ema(x) == ema((a*w1) @ x),
    so the DVE scan runs on a 32-row projected sequence, not [512, T].
  - Both batches stack in PSUM partitions (b0 rows 0-31, b1 rows 32-63 via
    PE tile_position), so ONE scan / ONE relu instruction covers both
    batches -- the scan is sequential per column, rows ride free.
  - b1 rides the DVE relu (fused add+max tensor_scalar); b2 rides the
    sigmoid ACTIVATE's per-partition bias.  No ones-row needed.
  - The gate multiply writes a fresh fp16 tile (not in-place) aiming for
    the DVE 2x packed 16-bit path; stores stream per cb-pair so the tail
    store starts right after its sigmoid.
  - All x loads AND stores issue on the Sync ring (HWDGE); weights/consts
    load via the Scalar ring so the first x load starts at t=0.
"""

import numpy as np
from contextlib import ExitStack

import concourse.bass as bass
import concourse.bacc as bacc
import concourse.tile as tile
import concourse.mybir as mybir
from concourse.bass_utils import run_bass_kernel_spmd

F32 = mybir.dt.float32
F16 = mybir.dt.float16

N_CORES = 8
B, C, T = 16, 512, 4096
CSQ = 32          # squeeze dim
P = 128           # SBUF partitions
NCB = C // P      # channel blocks (4)
B_LOC = B // N_CORES          # batches per core (2)
M2 = B_LOC * CSQ  # stacked mm1 output rows (64)
Tc = 1024         # time chunk
NCI = T // Tc     # DRAM chunk blocks (4)
TS = 512          # matmul / scan sub-tile (one PSUM bank)
PREF = 2          # load prefetch distance, in chunks
# Time chunks: the final 1024 columns run as two 512-col chunks so the
# last chunk's serial mm1->scan->relu->mm2->sigmoid->gate chain is
# half as deep.
CHUNKS = [(0, 1024), (1024, 1024), (2048, 1024), (3072, 512), (3584, 512)]
NTH = len(CHUNKS)


def build_nc(B_loc, cw, C_=C, T_=T):
    assert B_loc == B_LOC
    d = 1.0 - 1.0 / cw
    assert sum(c[1] for c in CHUNKS) == T_ and all(
        c % TS == 0 for _, c in CHUNKS)

    nc = bacc.Bacc(trn_type="TRN2")
    # x/out DRAM layout: [p, b, ci, cb, t] (fp16).  One load per (b, ci)
    # is 128 x 8KB contiguous; one store per (b, ci, cb-pair) is 128 x 4KB.
    xin = nc.declare_dram_parameter("x", [P, B_loc * NCI * NCB * Tc], F16,
                                    isOutput=False)
    w1e = nc.declare_dram_parameter("w1e", [P, NCB * CSQ], F16, isOutput=False)
    w2d = nc.declare_dram_parameter("w2d", [M2, C_], F16, isOutput=False)
    b1d = nc.declare_dram_parameter("b1d", [M2, 1], F32, isOutput=False)
    b2e = nc.declare_dram_parameter("b2e", [P, NCB], F32, isOutput=False)
    out = nc.declare_dram_parameter("out", [P, B_loc * NCI * NCB * Tc], F16,
                                    isOutput=True)

    xv = xin.rearrange("p (b ci cb t) -> p b ci cb t", b=B_loc, ci=NCI, cb=NCB)
    ov = out.rearrange("p (b ci cb t) -> p b ci cb t", b=B_loc, ci=NCI, cb=NCB)

    with ExitStack() as ctx:
        tc = ctx.enter_context(tile.TileContext(nc))
        const = ctx.enter_context(tc.tile_pool(name="const", bufs=1))
        xpool = ctx.enter_context(
            tc.tile_pool(name="xp", bufs=2 * (PREF + 1) + 1))
        opool = ctx.enter_context(tc.tile_pool(name="op", bufs=4))
        gpool = ctx.enter_context(tc.tile_pool(name="gp", bufs=4))
        upool = ctx.enter_context(tc.tile_pool(name="up", bufs=3))
        hpool = ctx.enter_context(tc.tile_pool(name="hp", bufs=3))
        cpool = ctx.enter_context(tc.tile_pool(name="cp", bufs=2))
        php = ctx.enter_context(tc.tile_pool(name="php", bufs=2, space="PSUM"))
        pgp = ctx.enter_context(tc.tile_pool(name="pgp", bufs=3, space="PSUM"))

        # Consts ride the Scalar HWDGE ring so the Sync ring starts on x
        # immediately.
        w1_t = const.tile([P, NCB * CSQ], F16, tag="w1e")
        nc.scalar.dma_start(w1_t[:], w1e[:])
        w2_t = const.tile([M2, C_], F16, tag="w2d")
        nc.scalar.dma_start(w2_t[:], w2d[:])
        b1_t = const.tile([M2, 1], F32, tag="b1d")
        nc.scalar.dma_start(b1_t[:], b1d[:])
        b2_t = const.tile([P, NCB], F32, tag="b2e")
        nc.scalar.dma_start(b2_t[:], b2e[:])
        dconst = const.tile([M2, TS], F32, tag="dconst")
        nc.vector.memset(dconst[:], d)

        xts = {}

        def emit_loads(ci):
            t0, tcc = CHUNKS[ci]
            dci, dt0 = divmod(t0, Tc)
            tiles = []
            for b in range(B_loc):
                xt = xpool.tile([P, NCB * Tc], F16, tag="x", name=f"x{b}_{ci}")
                xw = xt[:].rearrange("p (cb t) -> p cb t", cb=NCB)
                nc.sync.dma_start(
                    xw[:, :, 0:tcc],
                    xv[:, b, dci, :, dt0:dt0 + tcc])
                tiles.append(xt)
            xts[ci] = tiles

        for ci in range(min(PREF, NTH)):
            emit_loads(ci)

        ph_pre = {}

        def phase1(ci):
            # mm1 for chunk ci, both batches stacked into one PSUM tile
            # (b0 -> rows 0-31, b1 -> rows 32-63 via PE tile placement).
            # Emitted one chunk ahead so the PE never sits behind a
            # relu-blocked mm2 while independent mm1 work exists.
            _, tcc = CHUNKS[ci]
            nts = tcc // TS
            xws_ = [xt[:].rearrange("p (cb t) -> p cb t", cb=NCB)
                    for xt in xts[ci]]
            phs = [None] * nts
            for ts in range(nts):
                ph = php.tile([M2, TS], F32, tag="ph")
                # Interleave the two batches' accumulation chains: b0 is
                # PE col-tile (0,0), b1 is (0,32) -- alternating their MMs
                # lets the two 128x32 tiles co-execute (~2x PE throughput).
                for cb in range(NCB):
                    for b in range(B_loc):
                        nc.tensor.matmul(
                            ph[b * CSQ:(b + 1) * CSQ, :],
                            w1_t[:, cb * CSQ:(cb + 1) * CSQ],
                            xws_[b][:, cb, ts * TS:(ts + 1) * TS],
                            start=(cb == 0), stop=(cb == NCB - 1))
                phs[ts] = ph
            ph_pre[ci] = phs

        phase1(0)
        carry = None
        for th in range(NTH):
            if th + PREF < NTH:
                emit_loads(th + PREF)
            if th + 1 < NTH:
                phase1(th + 1)
            t0, tcc = CHUNKS[th]
            dci, dt0 = divmod(t0, Tc)
            nts = tcc // TS
            xb = xts.pop(th)
            phs = ph_pre.pop(th)
            # Phase 2: one scan per TS sub-tile + one fused relu per chunk,
            # covering BOTH batches (stacked rows).
            ut = upool.tile([M2, Tc], F32, tag="u")
            for ts in range(nts):
                if th == 0 and ts == 0:
                    # u_0 = cw * p_0 makes y[0] = x[0] exact.
                    init = cpool.tile([M2, 1], F32, tag="c")
                    nc.scalar.mul(init[:], phs[ts][:, 0:1], float(cw))
                    init_ap = init[:]
                else:
                    init_ap = carry
                nc.vector.tensor_tensor_scan(
                    ut[:, ts * TS:(ts + 1) * TS], dconst[:],
                    phs[ts][:], init_ap,
                    mybir.AluOpType.mult, mybir.AluOpType.add)
                carry = ut[:, (ts + 1) * TS - 1:(ts + 1) * TS]
            # Fused (u + b1) -> max(., 0) on the DVE keeps ACT free for
            # sigmoids (ACT is the busiest compute engine at fp16 IO).
            ht = hpool.tile([M2, Tc], F16, tag="h")
            nc.vector.tensor_scalar(
                ht[:, 0:tcc], ut[:, 0:tcc], b1_t[:], 0.0,
                mybir.AluOpType.add, mybir.AluOpType.max)
            # Phase 3: mm2 + sigmoid per (b, cb); all time sub-tiles of
            # the chunk land in one PSUM tile -> one sigmoid each, with
            # b2 riding the ACTIVATE's per-partition bias.
            gts = [gpool.tile([P, NCB * Tc], F16, tag="g", name=f"g{b}")
                   for b in range(B_loc)]
            gws = [g[:].rearrange("p (cb t) -> p cb t", cb=NCB) for g in gts]
            # Interleave the two batches' mm2 streams: b0 reads h rows
            # 0-31 (PE row-tile T0), b1 rows 32-63 (T4) -- alternating
            # lets the two 32x128 tiles co-execute.
            for cb in range(NCB):
                pgs = [pgp.tile([P, Tc], F32, tag="pg", name=f"pg{b}")
                       for b in range(B_loc)]
                for ts in range(nts):
                    for b in range(B_loc):
                        nc.tensor.matmul(
                            pgs[b][:, ts * TS:(ts + 1) * TS],
                            w2_t[b * CSQ:(b + 1) * CSQ, cb * P:(cb + 1) * P],
                            ht[b * CSQ:(b + 1) * CSQ, ts * TS:(ts + 1) * TS],
                            start=True, stop=True)
                for b in range(B_loc):
                    nc.scalar.activation(
                        gws[b][:, cb, 0:tcc], pgs[b][:, 0:tcc],
                        mybir.ActivationFunctionType.Sigmoid,
                        bias=b2_t[:, cb:cb + 1])
            # Phase 4: gate multiply into a fresh fp16 tile (all-16-bit,
            # step-1, 4B-aligned -> DVE packed rate), one piece per
            # cb-pair so each store can stream as soon as its half is
            # gated.  Stores stay on the Sync ring with the loads.
            for b in range(B_loc):
                ot = opool.tile([P, NCB * Tc], F16, tag="o", name=f"o{b}")
                ow = ot[:].rearrange("p (cb t) -> p cb t", cb=NCB)
                xw = xb[b][:].rearrange("p (cb t) -> p cb t", cb=NCB)
                for cbp in range(0, NCB, 2):
                    nc.vector.tensor_mul(
                        ow[:, cbp:cbp + 2, 0:tcc],
                        xw[:, cbp:cbp + 2, 0:tcc],
                        gws[b][:, cbp:cbp + 2, 0:tcc])
                    nc.sync.dma_start(
                        ov[:, b, dci, cbp:cbp + 2, dt0:dt0 + tcc],
                        ow[:, cbp:cbp + 2, 0:tcc])
    nc.compile()
    return nc


def make_in_maps(x, w1, b1, w2, b2, cw, n_cores=N_CORES):
    """Host-side shard + weight prep. Returns per-core input maps."""
    a = 1.0 / cw
    C_ = w2.shape[0]
    b_loc = x.shape[0] // n_cores

    w1sT = (np.asarray(w1) * a).T.astype(np.float32)      # [C, CSQ]
    w1e = np.empty((P, NCB * CSQ), dtype=np.float16)
    for cb in range(NCB):
        w1e[:, cb * CSQ:(cb + 1) * CSQ] = w1sT[cb * P:(cb + 1) * P, :]

    w2d = np.empty((M2, C_), dtype=np.float16)
    for b in range(b_loc):
        w2d[b * CSQ:(b + 1) * CSQ, :] = np.asarray(w2).T

    b1d = np.empty((M2, 1), dtype=np.float32)
    for b in range(b_loc):
        b1d[b * CSQ:(b + 1) * CSQ, 0] = np.asarray(b1)

    b2e = np.asarray(b2).astype(np.float32).reshape(NCB, P).T.copy()

    # [B, C, T] -> per-core [P, b, ci, cb, t] fp16 (see build_nc).
    x16 = np.asarray(x).astype(np.float16)
    x16 = x16.reshape(n_cores, b_loc, NCB, P, NCI, Tc)
    x16 = np.ascontiguousarray(x16.transpose(0, 3, 1, 4, 2, 5))
    x16 = x16.reshape(n_cores, P, b_loc * NCI * NCB * Tc)

    return [
        {"x": x16[i], "w1e": w1e, "w2d": w2d, "b1d": b1d, "b2e": b2e}
        for i in range(n_cores)
    ]


def unshard_out(results, n_cores=N_CORES, b_loc=B_LOC):
    """Per-core [P, b*ci*cb*t] fp16 -> full [B, C, T] fp32."""
    o = np.stack([r["out"] for r in results], axis=0)
    o = o.reshape(n_cores, P, b_loc, NCI, NCB, Tc)
    o = o.transpose(0, 2, 4, 1, 3, 5)          # [core, b, cb, p, ci, t]
    return np.ascontiguousarray(o).reshape(B, C, T).astype(np.float32)


_NC_CACHE = {}


def kernel(x, w1, b1, w2, b2, context_window):
    cw = int(context_window)
    x = np.asarray(x)
    key = (cw, x.shape)
    if key not in _NC_CACHE:
        _NC_CACHE[key] = build_nc(x.shape[0] // N_CORES, cw)
    nc = _NC_CACHE[key]
    in_maps = make_in_maps(
        np.asarray(x), np.asarray(w1), np.asarray(b1),
        np.asarray(w2), np.asarray(b2), cw)
    res = run_bass_kernel_spmd(nc, in_maps, core_ids=list(range(N_CORES)))
    return unshard_out(res.results)
